# revision 11
# baseline (speedup 1.0000x reference)
"""AuxSpatialGather (per-class masked mean pooling) Trainium2 kernel.

Computes, per sample b:  ctx[k, c] = mean over pixels n with gt[n]==k of feats[c, n]
(classes with zero pixels get 0), returned as [B, C, K, 1] float32.

Strategy (8 NeuronCores, data-parallel over batch, 2 samples/core):
  - feats arrive channel-major [C, HW]; the PE matmul contracts over the
    partition dim, so feats must become pixel-major on chip. fp32 matmul on
    TRN2 runs at 1/4 rate, so we cast fp32->fp16 on DVE after a plain f32
    HWDGE load (SWDGE cast-DMA measured ~3x slower per SDMA engine), then
    PE-transpose PAIRS of fp16 pixels viewed as one f32 element (halves the
    transpose count), evacuate PSUM->SBUF, and run the one-hot matmul in fp16
    (two parity-split matmuls over a stride-2 rhs view) with fp32 PSUM
    accumulation. Only precision loss: fp16 input quantization.
  - one-hot weights are built on-chip from gt via is_equal against constants,
    in the pair-interleaved pixel order matching the transposes.
  - per-class counts via a free-dim reduce + ones-vector matmul; the final
    [19, 512] context is scaled by 1/max(cnt,1) and transposed to [512, 19].
"""

import numpy as np

NUM_CLASSES = 19
B, C, H, W = 16, 512, 128, 128
HW = H * W
N_CORES = 8
S = B // N_CORES  # samples per core
P = 128  # partitions

_compiled = None


def _build_nc(s=S, c=C, hw=HW, qw=4096):
    from concourse import bacc, mybir
    from concourse.tile import TileContext
    from concourse.masks import make_identity

    f32 = mybir.dt.float32
    f16 = mybir.dt.float16
    i32 = mybir.dt.int32
    K = NUM_CLASSES
    n_ci = c // P  # channel tiles (4)
    n_q = hw // qw  # n-chunks per sample (4)
    n_u = 4  # quarters per chunk
    n_ju = qw // (256 * n_u)  # pair-windows per quarter (4)
    n_j = qw // 256  # pair-windows (256 pixels) per chunk (16)
    n_t = hw // P  # 128-pixel weight columns per sample (128)

    nc = bacc.Bacc("TRN2", target_bir_lowering=False)
    feats = nc.dram_tensor("feats", [s, c, hw], f32, kind="ExternalInput")
    gt = nc.dram_tensor("gt_seg_map", [s, hw], i32, kind="ExternalInput")
    out = nc.dram_tensor("out", [s, c, K], f32, kind="ExternalOutput")

    with TileContext(nc) as tc:
        with (
            tc.tile_pool(name="const", bufs=1) as const_pool,
            tc.tile_pool(name="stage", bufs=4) as stage_pool,
            tc.tile_pool(name="chunks", bufs=3) as chunk_pool,
            tc.tile_pool(name="planes", bufs=2) as plane_pool,
            tc.tile_pool(name="ft", bufs=4) as ft_pool,
            tc.tile_pool(name="small", bufs=2) as small_pool,
            tc.tile_pool(name="ftp", bufs=3, space="PSUM") as ftp_pool,
            tc.tile_pool(name="accp", bufs=2, space="PSUM") as acc_pool,
            tc.tile_pool(name="tinyp", bufs=1, space="PSUM") as tiny_pool,
        ):
            ident32 = const_pool.tile([P, P], f32)
            make_identity(nc, ident32[:])
            ones16 = const_pool.tile([P, 1], f16)
            nc.vector.memset(ones16[:], 1.0)

            def load_chunks(si, q, split):
                """Issue f32 loads + per-quarter DVE casts for (si, q).

                split=True quarters the loads too (startup: first tiles become
                available after ~0.5MB instead of 8MB); otherwise one 2MB DMA
                per channel tile keeps the stream at full DMA efficiency.
                """
                uw = qw // n_u
                tiles = []
                if split:
                    for u in range(n_u):
                        for ci in range(n_ci):
                            tiles.append((ci, slice(u * uw, (u + 1) * uw)))
                else:
                    for ci in range(n_ci):
                        tiles.append((ci, slice(0, qw)))
                chs = [None] * n_ci
                for ci, sl in tiles:
                    if chs[ci] is None:
                        chs[ci] = chunk_pool.tile([P, qw], f16, name=f"ch{ci}")
                    st = stage_pool.tile([P, sl.stop - sl.start], f32, name="st")
                    nc.sync.dma_start(
                        out=st[:],
                        in_=feats[
                            si, ci * P : (ci + 1) * P,
                            q * qw + sl.start : q * qw + sl.stop,
                        ],
                    )
                    for u0 in range(sl.start // uw, sl.stop // uw):
                        usl = slice(u0 * uw, (u0 + 1) * uw)
                        nc.vector.tensor_copy(
                            chs[ci][:, usl],
                            st[:, usl.start - sl.start : usl.stop - sl.start],
                        )
                return chs

            def build_planes(si):
                """One-hot planes for sample si (pair-order pixel layout).

                pixel order: n = q*qw + u*(qw//n_u) + 8*p + 2*j + par
                (u quarter, j in [0,n_ju)) -> G[p, t], t = q*32 + u*8 + 2j+par:
                per-partition runs of 8 contiguous gt elements.
                """
                G_i = plane_pool.tile([P, n_t], i32, name="G_i")
                # second HWDGE ring (ACT): off the FIFO ring feeding feat loads
                nc.scalar.dma_start(
                    out=G_i[:].rearrange("p (q u r) -> p q u r", q=n_q, u=n_u),
                    in_=gt[si].rearrange(
                        "(q u p r) -> p q u r", q=n_q, u=n_u, p=P
                    ),
                )
                G_f = plane_pool.tile([P, n_t], f16, name="G_f")
                nc.vector.tensor_copy(G_f[:], G_i[:])
                # planes[p, k*n_t + t] = (gt[pix(p,t)] == k)
                planes = plane_pool.tile([P, K * n_t], f16, name="planes")
                for k in range(K):
                    nc.vector.tensor_scalar(
                        planes[:, k * n_t : (k + 1) * n_t],
                        G_f[:],
                        float(k),
                        None,
                        op0=mybir.AluOpType.is_equal,
                    )
                return planes

            def build_recip(planes):
                """Per-class counts -> reciprocal [K, 1] (emitted at finalize
                so the tiny count-matmul never blocks PE's transpose stream)."""
                partial = small_pool.tile([P, K], f32, name="partial")
                nc.vector.tensor_reduce(
                    partial[:],
                    planes[:].rearrange("p (k t) -> p k t", k=K),
                    axis=mybir.AxisListType.X,
                    op=mybir.AluOpType.add,
                )
                partial16 = small_pool.tile([P, K], f16, name="partial16")
                nc.vector.tensor_copy(partial16[:], partial[:])
                cnt_ps = tiny_pool.tile([1, K], f32, name="cnt_ps")
                nc.tensor.matmul(
                    cnt_ps[:], ones16[:], partial16[:], start=True, stop=True
                )
                # transpose [1,K] -> [K,1] via DVE 32x32 block transpose
                cnt_sq = small_pool.tile([32, 32], f32, name="cnt_sq")
                nc.vector.memset(cnt_sq[:], 0.0)
                nc.vector.tensor_copy(cnt_sq[:1, :K], cnt_ps[:])
                cnt_tr = small_pool.tile([32, 32], f32, name="cnt_tr")
                nc.vector.transpose(cnt_tr[:], cnt_sq[:])
                recip = small_pool.tile([K, 1], f32, name="recip")
                nc.vector.tensor_scalar_max(recip[:], cnt_tr[:K, :1], 1.0)
                nc.vector.reciprocal(recip[:], recip[:])
                return recip

            # gt first (tiny DMA, weights gate the first matmuls), then the
            # first feat chunk (halved loads -> earlier first tile)
            planes_cur = build_planes(0)
            pending = load_chunks(0, 0, split=True)

            # ---- main loop: load -> cast -> pair-transpose -> matmul ----
            for si in range(s):
                acc = acc_pool.tile([K, c], f32, name="acc")
                W_all = planes_cur[:].rearrange("p (k t) -> p t k", t=n_t)
                for q in range(n_q):
                    chs = pending
                    # prefetch next chunk; build next sample's weights between
                    # the last load of a sample and the first of the next
                    if q + 1 < n_q:
                        pending = load_chunks(si, q + 1, split=False)
                    elif si + 1 < s:
                        pending = load_chunks(si + 1, 0, split=False)
                        planes_next = build_planes(si + 1)
                    if q == 0:
                        recip = build_recip(planes_cur)
                    for j in range(n_j):
                        u, ju = divmod(j, n_ju)
                        ubase = u * (qw // (2 * n_u))  # quarter offset in pairs
                        # transpose 4 f32-viewed pair-tiles -> ftp [128, 512]
                        ftp = ftp_pool.tile([P, c], f32, name="ftp")
                        for ci in range(n_ci):
                            # stride-n_ju pair columns within quarter u: output
                            # partition p holds pixels u*uw + 8p + 2ju + par
                            nc.tensor.transpose(
                                ftp[:, ci * P : (ci + 1) * P],
                                chs[ci][:].bitcast(f32)[
                                    :, ubase + ju : ubase + ju + (P - 1) * n_ju + 1 : n_ju
                                ],
                                ident32[:],
                            )
                        fts = ft_pool.tile([P, 2 * c], f16, name="fts")
                        if j % 2 == 0:
                            nc.vector.tensor_copy(fts[:].bitcast(f32), ftp[:])
                        else:
                            nc.scalar.copy(fts[:].bitcast(f32), ftp[:])
                        fts_pairs = fts[:].rearrange("p (c two) -> p two c", two=2)
                        for par in range(2):
                            t = q * (n_j * 2) + u * (2 * n_ju) + 2 * ju + par
                            nc.tensor.matmul(
                                acc[:],
                                W_all[:, t, :],
                                fts_pairs[:, par, :],
                                start=(t == 0),
                                stop=(t == n_t - 1),
                            )

                # ---- normalize + emit [c, K] ----
                final = small_pool.tile([K, c], f32, name="final")
                nc.vector.tensor_scalar(
                    final[:],
                    acc[:],
                    recip[:, :1],
                    None,
                    op0=mybir.AluOpType.mult,
                )
                outT_ps = tiny_pool.tile([P, n_ci * K], f32, name="outT_ps")
                for ci in range(n_ci):
                    nc.tensor.transpose(
                        outT_ps[:, ci * K : (ci + 1) * K],
                        final[:K, ci * P : (ci + 1) * P],
                        ident32[:K, :K],
                    )
                outT = small_pool.tile([P, n_ci * K], f32, name="outT")
                nc.vector.tensor_copy(outT[:], outT_ps[:])
                # SWDGE: keep the HWDGE feat-load queue free of DMAs that
                # wait on compute (FIFO per issuing engine)
                nc.gpsimd.dma_start(
                    out=out[si].rearrange("(ci p) k -> p ci k", p=P),
                    in_=outT[:].rearrange("p (ci k) -> p ci k", k=K),
                )
                if si + 1 < s:
                    planes_cur = planes_next
    nc.compile()
    return nc


def _get_compiled():
    global _compiled
    if _compiled is None:
        _compiled = _build_nc()
    return _compiled


def kernel(feats, gt_seg_map):
    from concourse.bass_utils import run_bass_kernel_spmd

    feats = np.asarray(feats, dtype=np.float32).reshape(B, C, HW)
    gt = np.asarray(gt_seg_map).astype(np.int32).reshape(B, HW)

    nc = _get_compiled()
    in_maps = []
    for i in range(N_CORES):
        in_maps.append(
            {
                "feats": feats[i * S : (i + 1) * S],
                "gt_seg_map": gt[i * S : (i + 1) * S],
            }
        )
    res = run_bass_kernel_spmd(nc, in_maps, core_ids=list(range(N_CORES)))
    parts = [res.results[i]["out"] for i in range(N_CORES)]  # each [S, C, K]
    full = np.concatenate(parts, axis=0)  # [B, C, K]
    return full[..., None].astype(np.float32)  # [B, C, K, 1]
